# revision 6
# baseline (speedup 1.0000x reference)
"""ArcFace loss distributed Bass kernel for 8 TRN2 NeuronCores.

Strategy (class-parallel over the 100000-class dim, fp8 compute):
  - Host: pad classes 100000 -> 8*12544, transpose W shard to [D, C_shard] per
    core, gather W[target] rows (pure data movement; no arithmetic on host).
  - Device (SPMD, identical program on 8 cores):
      * normalize x rows (scale folded: xn*16), transpose to xnT, cast to fp8
        DoubleRow layout [128, 2, B]
      * stream WT tiles (f32 from HBM), cast to fp8e4 with *256 prescale
        (cos is scale-invariant: both prescales cancel through the norms)
      * cosine matmul in fp8 DoubleRow perf mode (2x PE throughput):
        cosT[c,b] psum tiles [128, 512]
      * per-class ||w||^2 via fp8 DoubleRow gram matmul; diag extracted with
        scalar_tensor_tensor against identity on the otherwise-idle GpSimd
      * exp split across two engines:
          - ScalarE native: E = exp(scale_c * cos_ps - 30), bf16 out
          - VectorE Schraudolph: E = bitcast_bf16(int16(cos_ps * A_c + B2)),
            a 2^x linear-mantissa approximation; rel err ~2e-2 ripple,
            mean-calibrated; final loss error ~5e-5 (validated vs reference)
      * sum over classes: ones-vector matmuls accumulating into a single
        PSUM bank across all 98 blocks (PE, fully pipelined with cos MMs)
      * target-logit correction computed densely for all 512 rows on every
        core from host-gathered W[target] (f32, exact), emitted late to fill
        engine gaps
      * AllGather the [1,512] partial sums, sum 8 rows, nll = 30 + ln(total)
        - S*phi, mean -> scalar
Fixed max shift of S=30 is used for the softmax (cos <= 1), so no running max
is needed: exp(S*cos - 30) never overflows in fp32/bf16.
"""

import math
from contextlib import ExitStack

import numpy as np

from concourse import bacc, masks, mybir, tile
from concourse.bass_utils import run_bass_kernel_spmd

N_CORES = 8
B = 512
D = 512
NCLASS = 100000
C_SHARD = NCLASS // N_CORES      # 12500
C_PAD = 12544                    # 98 * 128
S = 30.0
MARGIN = 0.5
COS_M = math.cos(MARGIN)
SIN_M = math.sin(MARGIN)
BIAS = -30.0                     # fixed log-sum-exp shift (= -S)

# fp8 prescales (cancel exactly through the norm computed from the same data)
WSCALE = 256.0
XSCALE = 16.0

# Schraudolph bf16 exp: e^x ~ bitcast_bf16(int16(x*L2E*128 + 16256 - C))
L2E = 1.4426950408889634
C_SCH = 7.42
B2_SCH = 16256.0 - C_SCH - 30.0 * 128.0 * L2E      # folds the -30 bias
# value emitted for padded (all-zero) classes: bitcast_bf16(int16(B2_SCH))
_pv = int(B2_SCH)  # 10708; round/trunc identical
V_PAD = (1.0 + (_pv & 127) / 128.0) * 2.0 ** ((_pv >> 7) - 127)

f32 = mybir.dt.float32
bf16 = mybir.dt.bfloat16
i16 = mybir.dt.int16
f8 = mybir.dt.float8e4
AF = mybir.ActivationFunctionType
ALU = mybir.AluOpType
AX = mybir.AxisListType
DR = mybir.MatmulPerfMode.DoubleRow

P = 128


def _pin_act_tables():
    """Force Exp and Ln onto the single natural_log_exp_and_others table set
    so walrus doesn't ping-pong ACT table loads between exp/ln sets."""
    import concourse.bacc as _bacc
    import concourse.hw_specs as _hw
    if getattr(_bacc, "_act_tables_pinned", False):
        return
    _orig = _hw.get_activation_tables

    def _pinned(arch):
        tabs = _orig(arch)
        both = {AF.Exp, AF.Ln}
        for name, fns in tabs.items():
            if name != "natural_log_exp_and_others":
                tabs[name] = fns - both
        return tabs

    _bacc.get_activation_tables = _pinned
    _bacc._act_tables_pinned = True


def _exp_on_dve(ck, n_blocks):
    """Block -> engine assignment for the exp stage. DVE also carries the
    gram-diag extraction (GpSimd can't read PSUM), so it gets ~1/4 of the
    exps. Last block fixed on DVE (pad correction constant V_PAD assumes
    Schraudolph for pad classes)."""
    if ck == n_blocks - 1:
        return True
    return ck % 4 == 1


def build_arcface_nc(c_pad=C_PAD, c_real=C_SHARD, n_cores=N_CORES,
                     tile_sizes=None):
    if tile_sizes is None:
        tile_sizes = ([512, 1280] + [1792] * ((c_pad - 1792) // 1792)
                      if c_pad > 1792 else [c_pad])
    assert sum(tile_sizes) == c_pad and all(t % P == 0 for t in tile_sizes)
    n_tiles = len(tile_sizes)
    c_tile_max = max(tile_sizes)
    n_blocks = c_pad // P            # 98
    n_dk = D // P                    # 4
    n_d2 = n_dk // 2                 # 2 DoubleRow K-groups
    n_bk = B // P
    pad_corr = float(n_cores * (c_pad - c_real)) * V_PAD

    _pin_act_tables()
    nc = bacc.Bacc("TRN2", target_bir_lowering=False, debug=False,
                   num_devices=n_cores)

    wt_ext = nc.dram_tensor("wt", [D, c_pad], f32, kind="ExternalInput")
    x_ext = nc.dram_tensor("x", [B, D], f32, kind="ExternalInput")
    wtg_ext = nc.dram_tensor("wtg", [B, D], f32, kind="ExternalInput")
    out_ext = nc.dram_tensor("out", [1, 1], f32, kind="ExternalOutput")

    with ExitStack() as ctx:
        tc = ctx.enter_context(tile.TileContext(nc))
        cpool = ctx.enter_context(tc.tile_pool(name="consts", bufs=1))
        xpool = ctx.enter_context(tc.tile_pool(name="xpool", bufs=1))
        sm = ctx.enter_context(tc.tile_pool(name="smalls", bufs=1))
        spool = ctx.enter_context(tc.tile_pool(name="spool", bufs=3))
        wtpool = ctx.enter_context(tc.tile_pool(name="wtpool", bufs=2))
        wbpool = ctx.enter_context(tc.tile_pool(name="wbpool", bufs=3))
        epool = ctx.enter_context(tc.tile_pool(name="epool", bufs=8))
        jpool = ctx.enter_context(tc.tile_pool(name="jpool", bufs=3))
        ps_c = ctx.enter_context(tc.tile_pool(name="ps_c", bufs=3, space="PSUM"))
        ps_g = ctx.enter_context(tc.tile_pool(name="ps_g", bufs=2, space="PSUM"))
        ps_s = ctx.enter_context(tc.tile_pool(name="ps_s", bufs=1, space="PSUM"))
        dram = ctx.enter_context(tc.tile_pool(name="dram", bufs=1, space="DRAM"))

        # ---- constants ----
        ident = cpool.tile([P, P], f32)
        masks.make_identity(nc, ident[:])
        ident_bf = cpool.tile([P, P], bf16)
        masks.make_identity(nc, ident_bf[:])
        ones_f = cpool.tile([P, 1], f32)
        nc.vector.memset(ones_f[:], 1.0)
        ones_bf = cpool.tile([P, 1], bf16)
        nc.vector.memset(ones_bf[:], 1.0)
        bias_m30 = cpool.tile([P, 1], f32)
        nc.vector.memset(bias_m30[:], BIAS)
        bias_lnA = cpool.tile([P, 1], f32)        # ln(S/XSCALE): ACT exp scale
        nc.vector.memset(bias_lnA[:], float(np.log(S / XSCALE)))
        bias_lnS = cpool.tile([P, 1], f32)        # ln(S*128*L2E/XSCALE): DVE
        nc.vector.memset(bias_lnS[:], float(np.log(S * 128.0 * L2E / XSCALE)))
        warm = cpool.tile([P, 1], f32)
        nc.scalar.activation(warm[:], ones_f[:], AF.Ln)

        # running Esum accumulator bank [1, B] (one long PSUM accum group)
        se_ps = ps_s.tile([1, B], f32, name="se_ps")

        xall = None
        xb = None
        wtb_t = {}
        scA_t = {}
        scS_t = {}

        def emit_tile_front(t, c0, ct):
            n_sub = ct // P
            wtt = [wtpool.tile([P, c_tile_max], f32, name=f"wtt{d}",
                               tag=f"wtt{d}") for d in range(n_dk)]
            for d in range(n_dk):
                nc.sync.dma_start(
                    out=wtt[d][:, :ct],
                    in_=wt_ext.ap()[d * P:(d + 1) * P, c0:c0 + ct])
            # fp8 DoubleRow layout: wtb8[d2][p, j, c] = WT[(2*d2+j)*128+p, c]
            wtb8 = [wbpool.tile([P, 2, c_tile_max], f8, name=f"wtb8_{d2}",
                                tag=f"wtb8_{d2}") for d2 in range(n_d2)]
            for d in range(n_dk):
                d2, j = divmod(d, 2)
                dst = wtb8[d2][:, j, :ct]
                if d == 0:
                    if t % 2 == 0:
                        nc.scalar.activation(dst, wtt[d][:, :ct], AF.Copy,
                                             scale=WSCALE)
                    else:
                        nc.vector.tensor_scalar(out=dst, in0=wtt[d][:, :ct],
                                                scalar1=WSCALE, scalar2=None,
                                                op0=ALU.mult)
                else:
                    nc.gpsimd.tensor_scalar(out=dst, in0=wtt[d][:, :ct],
                                            scalar1=WSCALE, scalar2=None,
                                            op0=ALU.mult)
            ssq = spool.tile([P, 16], f32, name="ssq")
            for s_i in range(n_sub):
                g_ps = ps_g.tile([P, P], f32, tag="g", name="g_ps")
                for d2 in range(n_d2):
                    blk = wtb8[d2][:, :, s_i * P:(s_i + 1) * P]
                    nc.tensor.matmul(g_ps[:], blk, blk, start=(d2 == 0),
                                     stop=(d2 == n_d2 - 1), perf_mode=DR)
                junk_c = jpool.tile([P, P], f32, tag="junkg", name="junk_c")
                nc.vector.scalar_tensor_tensor(
                    out=junk_c[:], in0=g_ps[:], scalar=1.0, in1=ident[:],
                    op0=ALU.mult, op1=ALU.mult, accum_out=ssq[:, s_i:s_i + 1])
            nc.gpsimd.tensor_scalar(out=ssq[:, :n_sub], in0=ssq[:, :n_sub],
                                    scalar1=1e-30, scalar2=None, op0=ALU.max)
            lnq = spool.tile([P, 16], f32, name="lnq")
            nc.scalar.activation(lnq[:, :n_sub], ssq[:, :n_sub], AF.Ln)
            scA = spool.tile([P, 16], f32, name="scA")
            nc.scalar.activation(scA[:, :n_sub], lnq[:, :n_sub], AF.Exp,
                                 bias=bias_lnA[:], scale=-0.5)
            scS = spool.tile([P, 16], f32, name="scS")
            nc.scalar.activation(scS[:, :n_sub], lnq[:, :n_sub], AF.Exp,
                                 bias=bias_lnS[:], scale=-0.5)
            wtb_t[t] = wtb8
            scA_t[t] = scA
            scS_t[t] = scS

        def emit_xall():
            nonlocal xall, xb
            xall = xpool.tile([P, n_bk * D], f32)
            nc.sync.dma_start(
                out=xall[:].rearrange("p (k d) -> p k d", k=n_bk),
                in_=x_ext.ap().rearrange("(k p) d -> p k d", k=n_bk))
            xb = [xall[:, k * D:(k + 1) * D] for k in range(n_bk)]

        def emit_tile_cos(t, ck0, ct):
            n_sub = ct // P
            wtb8 = wtb_t.pop(t)
            scA = scA_t.pop(t)
            scS = scS_t.pop(t)
            for s_i in range(n_sub):
                ck = ck0 + s_i
                cos_ps = ps_c.tile([P, B], f32, tag="cos", name="cos_ps")
                for d2 in range(n_d2):
                    nc.tensor.matmul(cos_ps[:],
                                     wtb8[d2][:, :, s_i * P:(s_i + 1) * P],
                                     xnt8[d2][:], start=(d2 == 0),
                                     stop=(d2 == n_d2 - 1), perf_mode=DR)
                e_sb = epool.tile([P, B], bf16, tag="e", name="e_sb")
                if _exp_on_dve(ck, n_blocks):
                    nc.vector.tensor_scalar(
                        out=e_sb[:].bitcast(i16), in0=cos_ps[:],
                        scalar1=scS[:, s_i:s_i + 1], scalar2=B2_SCH,
                        op0=ALU.mult, op1=ALU.add)
                else:
                    nc.scalar.activation(e_sb[:], cos_ps[:], AF.Exp,
                                         bias=bias_m30[:],
                                         scale=scA[:, s_i:s_i + 1])
                nc.tensor.matmul(se_ps[:], ones_bf[:], e_sb[:],
                                 start=(ck == 0), stop=(ck == n_blocks - 1))

        tile_c0 = np.cumsum([0] + tile_sizes).tolist()

        emit_xall()
        emit_tile_front(0, 0, tile_sizes[0])

        # ---- x norms + normalize(*16) + transpose + fp8 cast ----
        qx = sm.tile([P, n_bk], f32)
        for k in range(n_bk):
            junk_a = jpool.tile([P, D], f32, tag="junk", name="junk_a")
            nc.vector.scalar_tensor_tensor(
                out=junk_a[:], in0=xb[k], scalar=1.0, in1=xb[k],
                op0=ALU.mult, op1=ALU.mult, accum_out=qx[:, k:k + 1])
        rx = sm.tile([P, n_bk], f32)
        nc.scalar.activation(rx[:], qx[:], AF.Ln)
        # rx = XSCALE * rsqrt(qx)
        bias_lnX = cpool.tile([P, 1], f32)
        nc.vector.memset(bias_lnX[:], float(np.log(XSCALE)))
        nc.scalar.activation(rx[:], rx[:], AF.Exp, bias=bias_lnX[:],
                             scale=-0.5)

        xn = [xpool.tile([P, D], bf16, name=f"xn{k}") for k in range(n_bk)]
        for k in range(n_bk):
            nc.vector.tensor_scalar(out=xn[k][:], in0=xb[k],
                                    scalar1=rx[:, k:k + 1], scalar2=None,
                                    op0=ALU.mult)
        xnt = [xpool.tile([P, B], bf16, name=f"xnt{d}") for d in range(n_dk)]
        for k in range(n_bk):
            tp_ps = ps_c.tile([P, B], bf16, tag="cos", name=f"tp_ps{k}")
            for d in range(n_dk):
                nc.tensor.transpose(tp_ps[:, d * P:(d + 1) * P],
                                    xn[k][:, d * P:(d + 1) * P], ident_bf[:])
            for d in range(n_dk):
                nc.vector.tensor_copy(xnt[d][:, k * P:(k + 1) * P],
                                      tp_ps[:, d * P:(d + 1) * P])
        # fp8 DoubleRow layout for the moving operand
        xnt8 = [xpool.tile([P, 2, B], f8, name=f"xnt8_{d2}")
                for d2 in range(n_d2)]
        for d in range(n_dk):
            d2, j = divmod(d, 2)
            nc.vector.tensor_copy(xnt8[d2][:, j, :], xnt[d][:])

        # ---- main loop ----
        for t, ct in enumerate(tile_sizes):
            if t + 1 < n_tiles:
                emit_tile_front(t + 1, tile_c0[t + 1], tile_sizes[t + 1])
            emit_tile_cos(t, tile_c0[t] // P, ct)

        # ---- target margin terms (dense over all B rows, every core);
        # emitted late so it fills engine gaps near the end of the main loop
        wgall = xpool.tile([P, n_bk * D], f32)
        nc.sync.dma_start(
            out=wgall[:].rearrange("p (k d) -> p k d", k=n_bk),
            in_=wtg_ext.ap().rearrange("(k p) d -> p k d", k=n_bk))
        qw = sm.tile([P, n_bk], f32)
        pt = sm.tile([P, n_bk], f32)
        for k in range(n_bk):
            junk_b = jpool.tile([P, D], f32, tag="junk", name="junk_b")
            nc.vector.scalar_tensor_tensor(
                out=junk_b[:], in0=wgall[:, k * D:(k + 1) * D], scalar=1.0,
                in1=wgall[:, k * D:(k + 1) * D],
                op0=ALU.mult, op1=ALU.mult, accum_out=qw[:, k:k + 1])
            junk_d = jpool.tile([P, D], f32, tag="junk", name="junk_d")
            nc.vector.scalar_tensor_tensor(
                out=junk_d[:], in0=xb[k], scalar=1.0,
                in1=wgall[:, k * D:(k + 1) * D],
                op0=ALU.mult, op1=ALU.mult, accum_out=pt[:, k:k + 1])
        q = sm.tile([P, n_bk], f32)
        nc.vector.tensor_mul(q[:], qw[:], qx[:])
        nc.vector.tensor_scalar(out=q[:], in0=q[:], scalar1=1e-30,
                                scalar2=None, op0=ALU.max)
        rq = sm.tile([P, n_bk], f32)
        nc.scalar.activation(rq[:], q[:], AF.Ln)
        nc.scalar.activation(rq[:], rq[:], AF.Exp, scale=-0.5)
        cos_t = sm.tile([P, n_bk], f32)
        nc.vector.tensor_mul(cos_t[:], pt[:], rq[:])
        # sine = sqrt(max(1 - cos^2, eps))
        om = sm.tile([P, n_bk], f32)
        nc.vector.tensor_mul(om[:], cos_t[:], cos_t[:])
        nc.vector.tensor_scalar(out=om[:], in0=om[:], scalar1=-1.0,
                                scalar2=1.0, op0=ALU.mult, op1=ALU.add)
        nc.vector.tensor_scalar(out=om[:], in0=om[:], scalar1=1e-36,
                                scalar2=None, op0=ALU.max)
        sine = sm.tile([P, n_bk], f32)
        nc.scalar.activation(sine[:], om[:], AF.Ln)
        nc.scalar.activation(sine[:], sine[:], AF.Exp, scale=0.5)
        # phi = cos*COS_M - sine*SIN_M ; easy margin: cos>0 ? phi : cos
        tmp = sm.tile([P, n_bk], f32)
        nc.vector.tensor_scalar(out=tmp[:], in0=cos_t[:], scalar1=COS_M,
                                scalar2=None, op0=ALU.mult)
        phi = sm.tile([P, n_bk], f32)
        nc.vector.scalar_tensor_tensor(out=phi[:], in0=sine[:], scalar=-SIN_M,
                                       in1=tmp[:], op0=ALU.mult, op1=ALU.add)
        mask = sm.tile([P, n_bk], mybir.dt.uint8)
        nc.vector.tensor_scalar(out=mask[:], in0=cos_t[:], scalar1=0.0,
                                scalar2=None, op0=ALU.is_gt)
        phi_f = sm.tile([P, n_bk], f32)
        nc.vector.select(phi_f[:], mask[:], phi[:], cos_t[:])
        # delta = exp(S*phi_f - 30) - exp(S*cos_t - 30)
        e1 = sm.tile([P, n_bk], f32)
        nc.scalar.activation(e1[:], phi_f[:], AF.Exp, bias=bias_m30[:], scale=S)
        e2 = sm.tile([P, n_bk], f32)
        nc.scalar.activation(e2[:], cos_t[:], AF.Exp, bias=bias_m30[:], scale=S)
        delta = sm.tile([P, n_bk], f32)
        nc.vector.tensor_sub(delta[:], e1[:], e2[:])
        # flip delta/phi_f to [1, B] row layout (overlaps with main loop)
        dp_ps = ps_g.tile([1, B], f32, tag="g", name="dp_ps")
        pp_ps = ps_g.tile([1, B], f32, tag="g", name="pp_ps")
        for k in range(n_bk):
            nc.tensor.transpose(dp_ps[0:1, k * P:(k + 1) * P],
                                delta[:, k:k + 1], ident[:])
            nc.tensor.transpose(pp_ps[0:1, k * P:(k + 1) * P],
                                phi_f[:, k:k + 1], ident[:])
        delta_row = sm.tile([1, B], f32)
        nc.vector.tensor_copy(delta_row[:], dp_ps[:])
        phi_row = sm.tile([1, B], f32)
        nc.vector.tensor_copy(phi_row[:], pp_ps[:])

        # ---- collective: AllGather partial sums, local 8-row sum ----
        sumE_sb = sm.tile([1, B], f32)
        nc.vector.tensor_copy(sumE_sb[:], se_ps[:])
        cc_in = dram.tile([1, B], f32)
        cc_out = dram.tile([n_cores, B], f32)
        nc.sync.dma_start(out=cc_in[:], in_=sumE_sb[:])
        nc.gpsimd.collective_compute(
            "AllGather", ALU.bypass,
            replica_groups=[list(range(n_cores))],
            ins=[cc_in.opt()], outs=[cc_out.opt()])
        red = sm.tile([n_cores, B], f32)
        nc.sync.dma_start(out=red[:], in_=cc_out[:])
        tot_ps = ps_s.tile([1, B], f32, name="tot_ps")
        nc.tensor.matmul(tot_ps[:], ones_f[0:n_cores, :], red[:],
                         start=True, stop=True)

        # ---- tail (row layout): nll = 30 + ln(total) - S*phi ----
        total = sm.tile([1, B], f32)
        nc.vector.scalar_tensor_tensor(out=total[:], in0=tot_ps[:],
                                       scalar=-pad_corr, in1=delta_row[:],
                                       op0=ALU.add, op1=ALU.add)
        lnt = sm.tile([1, B], f32)
        nc.scalar.activation(lnt[:], total[:], AF.Ln)
        nc.vector.tensor_scalar(out=lnt[:], in0=lnt[:], scalar1=-BIAS,
                                scalar2=None, op0=ALU.add)
        nll = sm.tile([1, B], f32)
        nc.vector.scalar_tensor_tensor(out=nll[:], in0=phi_row[:], scalar=-S,
                                       in1=lnt[:], op0=ALU.mult, op1=ALU.add)
        nll1 = sm.tile([1, 1], f32)
        nc.vector.reduce_sum(nll1[:], nll[:], axis=AX.X)
        mean_sb = sm.tile([1, 1], f32)
        nc.vector.tensor_scalar(out=mean_sb[:], in0=nll1[:],
                                scalar1=1.0 / float(B), scalar2=None,
                                op0=ALU.mult)
        nc.sync.dma_start(out=out_ext.ap()[:, :], in_=mean_sb[:])

    nc.compile()
    return nc


def _shard_inputs(input, weight, target, c_pad=C_PAD, c_real=C_SHARD,
                  n_cores=N_CORES):
    """Host-side data layout only: shard, pad, transpose, gather."""
    x = np.ascontiguousarray(input, dtype=np.float32)
    w = np.asarray(weight, dtype=np.float32)
    tgt = np.asarray(target).astype(np.int64)
    wtg = np.ascontiguousarray(w[tgt])  # [B, D] gathered target rows
    in_maps = []
    for j in range(n_cores):
        shard = w[j * c_real:(j + 1) * c_real]          # [c_real, D]
        wt = np.zeros((D, c_pad), dtype=np.float32)
        wt[:, :c_real] = shard.T
        in_maps.append({"wt": wt, "x": x, "wtg": wtg})
    return in_maps


_NC_CACHE = {}


def kernel(input, weight, target, _trace=False, _trace_kwargs=None):
    key = "full"
    if key not in _NC_CACHE:
        _NC_CACHE[key] = build_arcface_nc()
    nc = _NC_CACHE[key]
    in_maps = _shard_inputs(input, weight, target)
    res = run_bass_kernel_spmd(nc, in_maps, core_ids=list(range(N_CORES)),
                               trace=_trace, **(_trace_kwargs or {}))
    out = np.float32(res.results[0]["out"][0, 0])
    kernel.last_results = res
    return np.asarray(out, dtype=np.float32).reshape(())


if __name__ == "__main__":
    rng = np.random.default_rng(0)
    x = rng.standard_normal((B, D)).astype(np.float32)
    w = rng.standard_normal((NCLASS, D)).astype(np.float32) * 0.01
    t = rng.integers(0, NCLASS, size=(B,)).astype(np.int64)
    print("out:", kernel(x, w, t))


# revision 12
# speedup vs baseline: 3.3635x; 3.3635x over previous
"""ArcFace loss distributed Bass kernel for 8 TRN2 NeuronCores.

Strategy (class-parallel over the 100000-class dim, fp8 compute):
  - Host: pad classes 100000 -> 8*12544, transpose W shard to [D, C_shard] per
    core, gather W[target] rows (pure data movement; no arithmetic on host).
  - Device (SPMD, identical program on 8 cores):
      * normalize x rows (scale folded: xn*16), transpose to xnT, cast to fp8
        DoubleRow layout [128, 2, B]
      * stream WT tiles (f32 from HBM), cast to fp8e4 with *256 prescale
        (cos is scale-invariant: both prescales cancel through the norms)
      * cosine matmul in fp8 DoubleRow perf mode (2x PE throughput):
        cosT[c,b] psum tiles [128, 512]
      * per-class ||w||^2 via fp8 DoubleRow gram matmul; diag extracted with
        scalar_tensor_tensor against identity on the otherwise-idle GpSimd
      * exp split across two engines:
          - ScalarE native: E = exp(scale_c * cos_ps - 30), bf16 out
          - VectorE Schraudolph: E = bitcast_bf16(int16(cos_ps * A_c + B2)),
            a 2^x linear-mantissa approximation; rel err ~2e-2 ripple,
            mean-calibrated; final loss error ~5e-5 (validated vs reference)
      * sum over classes: ones-vector matmuls accumulating into a single
        PSUM bank across all 98 blocks (PE, fully pipelined with cos MMs)
      * target-logit correction computed densely for all 512 rows on every
        core from host-gathered W[target] (f32, exact), emitted late to fill
        engine gaps
      * AllGather the [1,512] partial sums, sum 8 rows, nll = 30 + ln(total)
        - S*phi, mean -> scalar
Fixed max shift of S=30 is used for the softmax (cos <= 1), so no running max
is needed: exp(S*cos - 30) never overflows in fp32/bf16.
"""

import math
from contextlib import ExitStack

import numpy as np

from concourse import bacc, masks, mybir, tile
from concourse.bass_utils import run_bass_kernel_spmd

N_CORES = 8
B = 512
D = 512
NCLASS = 100000
C_SHARD = NCLASS // N_CORES      # 12500
C_PAD = 12544                    # 98 * 128
S = 30.0
MARGIN = 0.5
COS_M = math.cos(MARGIN)
SIN_M = math.sin(MARGIN)
BIAS = -30.0                     # fixed log-sum-exp shift (= -S)

# fp8 prescales (cancel exactly through the norm computed from the same data)
WSCALE = 256.0
XSCALE = 16.0

# Schraudolph bf16 exp: e^x ~ bitcast_bf16(int16(x*L2E*128 + 16256 - C))
L2E = 1.4426950408889634
C_SCH = 7.42
B2_SCH = 16256.0 - C_SCH - 30.0 * 128.0 * L2E      # folds the -30 bias
# value emitted for padded (all-zero) classes: bitcast_bf16(int16(B2_SCH))
_pv = int(B2_SCH)  # 10708; round/trunc identical
V_PAD = (1.0 + (_pv & 127) / 128.0) * 2.0 ** ((_pv >> 7) - 127)

f32 = mybir.dt.float32
bf16 = mybir.dt.bfloat16
i16 = mybir.dt.int16
f8 = mybir.dt.float8e4
AF = mybir.ActivationFunctionType
ALU = mybir.AluOpType
AX = mybir.AxisListType
DR = mybir.MatmulPerfMode.DoubleRow

P = 128


def _pin_act_tables():
    """Force Exp and Ln onto the single natural_log_exp_and_others table set
    so walrus doesn't ping-pong ACT table loads between exp/ln sets."""
    import concourse.bacc as _bacc
    import concourse.hw_specs as _hw
    if getattr(_bacc, "_act_tables_pinned", False):
        return
    _orig = _hw.get_activation_tables

    def _pinned(arch):
        tabs = _orig(arch)
        both = {AF.Exp, AF.Ln}
        for name, fns in tabs.items():
            if name != "natural_log_exp_and_others":
                tabs[name] = fns - both
        return tabs

    _bacc.get_activation_tables = _pinned
    _bacc._act_tables_pinned = True


def _exp_on_dve(ck, n_blocks):
    """Block -> engine assignment for the exp stage. ACT is the only engine
    with fast fp8 conversion so it carries all W casts; DVE (which also runs
    the gram-diag extraction) takes ~2/3 of the exps. Last block fixed on DVE
    (pad correction constant V_PAD assumes Schraudolph for pad classes)."""
    if ck == n_blocks - 1:
        return True
    return ck % 14 >= 5


def build_arcface_nc(c_pad=C_PAD, c_real=C_SHARD, n_cores=N_CORES,
                     tile_sizes=None):
    if tile_sizes is None:
        tile_sizes = ([512, 1280] + [1792] * ((c_pad - 1792) // 1792)
                      if c_pad > 1792 else [c_pad])
    assert sum(tile_sizes) == c_pad and all(t % P == 0 for t in tile_sizes)
    n_tiles = len(tile_sizes)
    c_tile_max = max(tile_sizes)
    n_blocks = c_pad // P            # 98
    n_dk = D // P                    # 4
    n_d2 = n_dk // 2                 # 2 DoubleRow K-groups
    n_bk = B // P
    pad_corr = float(n_cores * (c_pad - c_real)) * V_PAD

    _pin_act_tables()
    nc = bacc.Bacc("TRN2", target_bir_lowering=False, debug=False,
                   num_devices=n_cores)

    wt_ext = nc.dram_tensor("wt", [D, c_pad], f32, kind="ExternalInput")
    x_ext = nc.dram_tensor("x", [B, D], f32, kind="ExternalInput")
    wtg_ext = nc.dram_tensor("wtg", [B, D], f32, kind="ExternalInput")
    out_ext = nc.dram_tensor("out", [1, 1], f32, kind="ExternalOutput")

    with ExitStack() as ctx:
        tc = ctx.enter_context(tile.TileContext(nc))
        cpool = ctx.enter_context(tc.tile_pool(name="consts", bufs=1))
        xpool = ctx.enter_context(tc.tile_pool(name="xpool", bufs=1))
        sm = ctx.enter_context(tc.tile_pool(name="smalls", bufs=1))
        spool = ctx.enter_context(tc.tile_pool(name="spool", bufs=3))
        wtpool = ctx.enter_context(tc.tile_pool(name="wtpool", bufs=2))
        wbpool = ctx.enter_context(tc.tile_pool(name="wbpool", bufs=3))
        epool = ctx.enter_context(tc.tile_pool(name="epool", bufs=8))
        jpool = ctx.enter_context(tc.tile_pool(name="jpool", bufs=3))
        ps_c = ctx.enter_context(tc.tile_pool(name="ps_c", bufs=3, space="PSUM"))
        ps_g = ctx.enter_context(tc.tile_pool(name="ps_g", bufs=2, space="PSUM"))
        ps_s = ctx.enter_context(tc.tile_pool(name="ps_s", bufs=1, space="PSUM"))
        dram = ctx.enter_context(tc.tile_pool(name="dram", bufs=1, space="DRAM"))

        # ---- constants ----
        ident = cpool.tile([P, P], f32)
        masks.make_identity(nc, ident[:])
        ident_bf = cpool.tile([P, P], bf16)
        masks.make_identity(nc, ident_bf[:])
        ones_f = cpool.tile([P, 1], f32)
        nc.vector.memset(ones_f[:], 1.0)
        ones_bf = cpool.tile([P, 1], bf16)
        nc.vector.memset(ones_bf[:], 1.0)
        bias_m30 = cpool.tile([P, 1], f32)
        nc.vector.memset(bias_m30[:], BIAS)
        bias_lnA = cpool.tile([P, 1], f32)        # ln(S/XSCALE): ACT exp scale
        nc.vector.memset(bias_lnA[:], float(np.log(S / XSCALE)))
        bias_lnS = cpool.tile([P, 1], f32)        # ln(S*128*L2E/XSCALE): DVE
        nc.vector.memset(bias_lnS[:], float(np.log(S * 128.0 * L2E / XSCALE)))
        warm = cpool.tile([P, 1], f32)
        nc.scalar.activation(warm[:], ones_f[:], AF.Ln)

        # running Esum accumulator bank [1, B] (one long PSUM accum group)
        se_ps = ps_s.tile([1, B], f32, name="se_ps")

        xall = None
        xb = None
        wtb_t = {}
        scA_t = {}
        scS_t = {}

        def emit_tile_front(t, c0, ct):
            n_sub = ct // P
            wtt = [wtpool.tile([P, c_tile_max], f32, name=f"wtt{d}",
                               tag=f"wtt{d}") for d in range(n_dk)]
            for d in range(n_dk):
                nc.sync.dma_start(
                    out=wtt[d][:, :ct],
                    in_=wt_ext.ap()[d * P:(d + 1) * P, c0:c0 + ct])
            # fp8 DoubleRow layout: wtb8[d2][p, j, c] = WT[(2*d2+j)*128+p, c]
            wtb8 = [wbpool.tile([P, 2, c_tile_max], f8, name=f"wtb8_{d2}",
                                tag=f"wtb8_{d2}") for d2 in range(n_d2)]
            # fp8 conversion is only fast on the ACT engine
            for d in range(n_dk):
                d2, j = divmod(d, 2)
                dst = wtb8[d2][:, j, :ct]
                nc.scalar.activation(dst, wtt[d][:, :ct], AF.Copy,
                                     scale=WSCALE)
            ssq = spool.tile([P, 16], f32, name="ssq")
            for s_i in range(n_sub):
                g_ps = ps_g.tile([P, P], f32, tag="g", name="g_ps")
                for d2 in range(n_d2):
                    blk = wtb8[d2][:, :, s_i * P:(s_i + 1) * P]
                    nc.tensor.matmul(g_ps[:], blk, blk, start=(d2 == 0),
                                     stop=(d2 == n_d2 - 1), perf_mode=DR)
                junk_c = jpool.tile([P, P], f32, tag="junkg", name="junk_c")
                nc.vector.scalar_tensor_tensor(
                    out=junk_c[:], in0=g_ps[:], scalar=1.0, in1=ident[:],
                    op0=ALU.mult, op1=ALU.mult, accum_out=ssq[:, s_i:s_i + 1])
            nc.gpsimd.tensor_scalar(out=ssq[:, :n_sub], in0=ssq[:, :n_sub],
                                    scalar1=1e-30, scalar2=None, op0=ALU.max)
            lnq = spool.tile([P, 16], f32, name="lnq")
            nc.scalar.activation(lnq[:, :n_sub], ssq[:, :n_sub], AF.Ln)
            scA = spool.tile([P, 16], f32, name="scA")
            nc.scalar.activation(scA[:, :n_sub], lnq[:, :n_sub], AF.Exp,
                                 bias=bias_lnA[:], scale=-0.5)
            scS = spool.tile([P, 16], f32, name="scS")
            nc.scalar.activation(scS[:, :n_sub], lnq[:, :n_sub], AF.Exp,
                                 bias=bias_lnS[:], scale=-0.5)
            wtb_t[t] = wtb8
            scA_t[t] = scA
            scS_t[t] = scS

        def emit_xall():
            nonlocal xall, xb
            xall = xpool.tile([P, n_bk * D], f32)
            nc.sync.dma_start(
                out=xall[:].rearrange("p (k d) -> p k d", k=n_bk),
                in_=x_ext.ap().rearrange("(k p) d -> p k d", k=n_bk))
            xb = [xall[:, k * D:(k + 1) * D] for k in range(n_bk)]

        def emit_tile_cos(t, ck0, ct):
            n_sub = ct // P
            wtb8 = wtb_t.pop(t)
            scA = scA_t.pop(t)
            scS = scS_t.pop(t)
            for s_i in range(n_sub):
                ck = ck0 + s_i
                cos_ps = ps_c.tile([P, B], f32, tag="cos", name="cos_ps")
                for d2 in range(n_d2):
                    nc.tensor.matmul(cos_ps[:],
                                     wtb8[d2][:, :, s_i * P:(s_i + 1) * P],
                                     xnt8[d2][:], start=(d2 == 0),
                                     stop=(d2 == n_d2 - 1), perf_mode=DR)
                e_sb = epool.tile([P, B], bf16, tag="e", name="e_sb")
                if _exp_on_dve(ck, n_blocks):
                    nc.vector.tensor_scalar(
                        out=e_sb[:].bitcast(i16), in0=cos_ps[:],
                        scalar1=scS[:, s_i:s_i + 1], scalar2=B2_SCH,
                        op0=ALU.mult, op1=ALU.add)
                else:
                    nc.scalar.activation(e_sb[:], cos_ps[:], AF.Exp,
                                         bias=bias_m30[:],
                                         scale=scA[:, s_i:s_i + 1])
                nc.tensor.matmul(se_ps[:], ones_bf[:], e_sb[:],
                                 start=(ck == 0), stop=(ck == n_blocks - 1))

        tile_c0 = np.cumsum([0] + tile_sizes).tolist()

        emit_xall()
        emit_tile_front(0, 0, tile_sizes[0])

        # ---- x norms + normalize(*16) + transpose + fp8 cast ----
        qx = sm.tile([P, n_bk], f32)
        for k in range(n_bk):
            junk_a = jpool.tile([P, D], f32, tag="junk", name="junk_a")
            nc.vector.scalar_tensor_tensor(
                out=junk_a[:], in0=xb[k], scalar=1.0, in1=xb[k],
                op0=ALU.mult, op1=ALU.mult, accum_out=qx[:, k:k + 1])
        rx = sm.tile([P, n_bk], f32)
        nc.scalar.activation(rx[:], qx[:], AF.Ln)
        # rx = XSCALE * rsqrt(qx)
        bias_lnX = cpool.tile([P, 1], f32)
        nc.vector.memset(bias_lnX[:], float(np.log(XSCALE)))
        nc.scalar.activation(rx[:], rx[:], AF.Exp, bias=bias_lnX[:],
                             scale=-0.5)

        xn = [xpool.tile([P, D], bf16, name=f"xn{k}") for k in range(n_bk)]
        for k in range(n_bk):
            nc.vector.tensor_scalar(out=xn[k][:], in0=xb[k],
                                    scalar1=rx[:, k:k + 1], scalar2=None,
                                    op0=ALU.mult)
        xnt = [xpool.tile([P, B], bf16, name=f"xnt{d}") for d in range(n_dk)]
        for k in range(n_bk):
            tp_ps = ps_c.tile([P, B], bf16, tag="cos", name=f"tp_ps{k}")
            for d in range(n_dk):
                nc.tensor.transpose(tp_ps[:, d * P:(d + 1) * P],
                                    xn[k][:, d * P:(d + 1) * P], ident_bf[:])
            for d in range(n_dk):
                nc.vector.tensor_copy(xnt[d][:, k * P:(k + 1) * P],
                                      tp_ps[:, d * P:(d + 1) * P])
        # fp8 DoubleRow layout for the moving operand (ACT: fast fp8 convert)
        xnt8 = [xpool.tile([P, 2, B], f8, name=f"xnt8_{d2}")
                for d2 in range(n_d2)]
        for d in range(n_dk):
            d2, j = divmod(d, 2)
            nc.scalar.activation(xnt8[d2][:, j, :], xnt[d][:], AF.Copy)

        # ---- main loop ----
        for t, ct in enumerate(tile_sizes):
            if t + 1 < n_tiles:
                emit_tile_front(t + 1, tile_c0[t + 1], tile_sizes[t + 1])
            emit_tile_cos(t, tile_c0[t] // P, ct)

        # ---- target margin terms (dense over all B rows, every core);
        # emitted late so it fills engine gaps near the end of the main loop
        wgall = xpool.tile([P, n_bk * D], f32)
        nc.sync.dma_start(
            out=wgall[:].rearrange("p (k d) -> p k d", k=n_bk),
            in_=wtg_ext.ap().rearrange("(k p) d -> p k d", k=n_bk))
        qw = sm.tile([P, n_bk], f32)
        pt = sm.tile([P, n_bk], f32)
        for k in range(n_bk):
            junk_b = jpool.tile([P, D], f32, tag="junk", name="junk_b")
            nc.vector.scalar_tensor_tensor(
                out=junk_b[:], in0=wgall[:, k * D:(k + 1) * D], scalar=1.0,
                in1=wgall[:, k * D:(k + 1) * D],
                op0=ALU.mult, op1=ALU.mult, accum_out=qw[:, k:k + 1])
            junk_d = jpool.tile([P, D], f32, tag="junk", name="junk_d")
            nc.vector.scalar_tensor_tensor(
                out=junk_d[:], in0=xb[k], scalar=1.0,
                in1=wgall[:, k * D:(k + 1) * D],
                op0=ALU.mult, op1=ALU.mult, accum_out=pt[:, k:k + 1])
        q = sm.tile([P, n_bk], f32)
        nc.vector.tensor_mul(q[:], qw[:], qx[:])
        nc.vector.tensor_scalar(out=q[:], in0=q[:], scalar1=1e-30,
                                scalar2=None, op0=ALU.max)
        rq = sm.tile([P, n_bk], f32)
        nc.scalar.activation(rq[:], q[:], AF.Ln)
        nc.scalar.activation(rq[:], rq[:], AF.Exp, scale=-0.5)
        cos_t = sm.tile([P, n_bk], f32)
        nc.vector.tensor_mul(cos_t[:], pt[:], rq[:])
        # sine = sqrt(max(1 - cos^2, eps))
        om = sm.tile([P, n_bk], f32)
        nc.vector.tensor_mul(om[:], cos_t[:], cos_t[:])
        nc.vector.tensor_scalar(out=om[:], in0=om[:], scalar1=-1.0,
                                scalar2=1.0, op0=ALU.mult, op1=ALU.add)
        nc.vector.tensor_scalar(out=om[:], in0=om[:], scalar1=1e-36,
                                scalar2=None, op0=ALU.max)
        sine = sm.tile([P, n_bk], f32)
        nc.scalar.activation(sine[:], om[:], AF.Ln)
        nc.scalar.activation(sine[:], sine[:], AF.Exp, scale=0.5)
        # phi = cos*COS_M - sine*SIN_M ; easy margin: cos>0 ? phi : cos
        tmp = sm.tile([P, n_bk], f32)
        nc.vector.tensor_scalar(out=tmp[:], in0=cos_t[:], scalar1=COS_M,
                                scalar2=None, op0=ALU.mult)
        phi = sm.tile([P, n_bk], f32)
        nc.vector.scalar_tensor_tensor(out=phi[:], in0=sine[:], scalar=-SIN_M,
                                       in1=tmp[:], op0=ALU.mult, op1=ALU.add)
        mask = sm.tile([P, n_bk], mybir.dt.uint8)
        nc.vector.tensor_scalar(out=mask[:], in0=cos_t[:], scalar1=0.0,
                                scalar2=None, op0=ALU.is_gt)
        phi_f = sm.tile([P, n_bk], f32)
        nc.vector.select(phi_f[:], mask[:], phi[:], cos_t[:])
        # delta = exp(S*phi_f - 30) - exp(S*cos_t - 30)
        e1 = sm.tile([P, n_bk], f32)
        nc.scalar.activation(e1[:], phi_f[:], AF.Exp, bias=bias_m30[:], scale=S)
        e2 = sm.tile([P, n_bk], f32)
        nc.scalar.activation(e2[:], cos_t[:], AF.Exp, bias=bias_m30[:], scale=S)
        delta = sm.tile([P, n_bk], f32)
        nc.vector.tensor_sub(delta[:], e1[:], e2[:])
        # flip delta/phi_f to [1, B] row layout (overlaps with main loop)
        dp_ps = ps_g.tile([1, B], f32, tag="g", name="dp_ps")
        pp_ps = ps_g.tile([1, B], f32, tag="g", name="pp_ps")
        for k in range(n_bk):
            nc.tensor.transpose(dp_ps[0:1, k * P:(k + 1) * P],
                                delta[:, k:k + 1], ident[:])
            nc.tensor.transpose(pp_ps[0:1, k * P:(k + 1) * P],
                                phi_f[:, k:k + 1], ident[:])
        delta_row = sm.tile([1, B], f32)
        nc.vector.tensor_copy(delta_row[:], dp_ps[:])
        phi_row = sm.tile([1, B], f32)
        nc.vector.tensor_copy(phi_row[:], pp_ps[:])

        # ---- collective: AllGather partial sums, local 8-row sum ----
        sumE_sb = sm.tile([1, B], f32)
        nc.vector.tensor_copy(sumE_sb[:], se_ps[:])
        cc_in = dram.tile([1, B], f32)
        cc_out = dram.tile([n_cores, B], f32)
        nc.sync.dma_start(out=cc_in[:], in_=sumE_sb[:])
        nc.gpsimd.collective_compute(
            "AllGather", ALU.bypass,
            replica_groups=[list(range(n_cores))],
            ins=[cc_in.opt()], outs=[cc_out.opt()])
        red = sm.tile([n_cores, B], f32)
        nc.sync.dma_start(out=red[:], in_=cc_out[:])
        tot_ps = ps_s.tile([1, B], f32, name="tot_ps")
        nc.tensor.matmul(tot_ps[:], ones_f[0:n_cores, :], red[:],
                         start=True, stop=True)

        # ---- tail (row layout): nll = 30 + ln(total) - S*phi ----
        total = sm.tile([1, B], f32)
        nc.vector.scalar_tensor_tensor(out=total[:], in0=tot_ps[:],
                                       scalar=-pad_corr, in1=delta_row[:],
                                       op0=ALU.add, op1=ALU.add)
        lnt = sm.tile([1, B], f32)
        nc.scalar.activation(lnt[:], total[:], AF.Ln)
        nc.vector.tensor_scalar(out=lnt[:], in0=lnt[:], scalar1=-BIAS,
                                scalar2=None, op0=ALU.add)
        nll = sm.tile([1, B], f32)
        nc.vector.scalar_tensor_tensor(out=nll[:], in0=phi_row[:], scalar=-S,
                                       in1=lnt[:], op0=ALU.mult, op1=ALU.add)
        nll1 = sm.tile([1, 1], f32)
        nc.vector.reduce_sum(nll1[:], nll[:], axis=AX.X)
        mean_sb = sm.tile([1, 1], f32)
        nc.vector.tensor_scalar(out=mean_sb[:], in0=nll1[:],
                                scalar1=1.0 / float(B), scalar2=None,
                                op0=ALU.mult)
        nc.sync.dma_start(out=out_ext.ap()[:, :], in_=mean_sb[:])

    nc.compile()
    return nc


def _shard_inputs(input, weight, target, c_pad=C_PAD, c_real=C_SHARD,
                  n_cores=N_CORES):
    """Host-side data layout only: shard, pad, transpose, gather."""
    x = np.ascontiguousarray(input, dtype=np.float32)
    w = np.asarray(weight, dtype=np.float32)
    tgt = np.asarray(target).astype(np.int64)
    wtg = np.ascontiguousarray(w[tgt])  # [B, D] gathered target rows
    in_maps = []
    for j in range(n_cores):
        shard = w[j * c_real:(j + 1) * c_real]          # [c_real, D]
        wt = np.zeros((D, c_pad), dtype=np.float32)
        wt[:, :c_real] = shard.T
        in_maps.append({"wt": wt, "x": x, "wtg": wtg})
    return in_maps


_NC_CACHE = {}


def kernel(input, weight, target, _trace=False, _trace_kwargs=None):
    key = "full"
    if key not in _NC_CACHE:
        _NC_CACHE[key] = build_arcface_nc()
    nc = _NC_CACHE[key]
    in_maps = _shard_inputs(input, weight, target)
    res = run_bass_kernel_spmd(nc, in_maps, core_ids=list(range(N_CORES)),
                               trace=_trace, **(_trace_kwargs or {}))
    out = np.float32(res.results[0]["out"][0, 0])
    kernel.last_results = res
    return np.asarray(out, dtype=np.float32).reshape(())


if __name__ == "__main__":
    rng = np.random.default_rng(0)
    x = rng.standard_normal((B, D)).astype(np.float32)
    w = rng.standard_normal((NCLASS, D)).astype(np.float32) * 0.01
    t = rng.integers(0, NCLASS, size=(B,)).astype(np.int64)
    print("out:", kernel(x, w, t))
